# revision 32
# baseline (speedup 1.0000x reference)
"""DynamicGraphEmbedding kernel for 8 Trainium2 NeuronCores.

The reference collapses algebraically:
  - deg[i] == K == 16 for every node (dst list is repeat(arange(N), K)),
    so gcn_norm edge weight ew == 1/16 for every edge.
  - straight-through gumbel gate is exactly y_hard in the forward pass,
    i.e. gate(e) = 1 iff argmax(softmax(logits[e] + g[e])) == 0.
  - therefore out[b] = A @ (x[b] @ W) + bias, with the dense [N, N] matrix
    A[i, j] = gate(i*N+j)/16 if j in topk_j[i] else 0.

Host (tiny, O(N^2)): build A from emb/logits/gumbel_u with the exact same
jax-on-CPU ops as the reference. Device (the memory-bound bulk): two chained
256^3 matmuls per batch element, data-parallel over batch across 8 cores.
"""

import sys

import numpy as np

if "/opt/trn_rl_repo" not in sys.path:
    sys.path.insert(0, "/opt/trn_rl_repo")

N, T, B, D, K = 256, 256, 64, 64, 16
NCORES = 8
BPC = B // NCORES  # batch elements per core

_CACHE = {}
LAST_RESULT = None  # BassKernelResults of the most recent run (for profiling)


def _graph_matrix(emb, logits, gumbel_u):
    """Dense [N, N] combined gate/topk/gcn-norm matrix A (host-side, tiny)."""
    try:
        import jax
        import jax.numpy as jnp

        cpu = jax.devices("cpu")[0]
        emb_j = jax.device_put(np.asarray(emb), cpu)
        logits_j = jax.device_put(np.asarray(logits), cpu)
        gu_j = jax.device_put(np.asarray(gumbel_u), cpu)
        nrm = jnp.linalg.norm(emb_j, axis=-1)
        cos = (emb_j @ emb_j.T) / (nrm[:, None] * nrm[None, :])
        _, topk_j = jax.lax.top_k(cos, K)
        g = -jnp.log(-jnp.log(gu_j))
        y_soft = jax.nn.softmax(logits_j + g, axis=-1)
        am = jnp.argmax(y_soft, axis=-1)
        topk = np.asarray(topk_j)
        gate_full = (np.asarray(am) == 0).astype(np.float32)
    except Exception:
        emb32 = np.asarray(emb, np.float32)
        nrm = np.sqrt((emb32 * emb32).sum(-1))
        cos = (emb32 @ emb32.T) / (nrm[:, None] * nrm[None, :])
        topk = np.argsort(-cos, axis=-1, kind="stable")[:, :K]
        lg = np.asarray(logits, np.float32) + np.float32(-1.0) * np.log(
            -np.log(np.asarray(gumbel_u, np.float32))
        )
        e = np.exp(lg - lg.max(-1, keepdims=True))
        y_soft = e / e.sum(-1, keepdims=True)
        gate_full = (np.argmax(y_soft, -1) == 0).astype(np.float32)
    rows = np.repeat(np.arange(N), K)
    cols = topk.reshape(-1)
    A = np.zeros((N, N), np.float32)
    A[rows, cols] = gate_full[rows * N + cols] * np.float32(0.0625)
    return A


NG = BPC // 2  # batch pairs per core


def _build_bass(with_bias):
    """Per-core Bass graph: out[b] = A @ (x[b] @ W) [+ bias] for BPC batches.

    Host-packed layouts (8KB contiguous per-partition runs, few big DMAs):
      consts [128, 4, 256]        [p, g, t]: g = (W c0, W c1, AT c0, AT c1)
      xin    [NG, 128, 2, 2, 256] [g, p, c, bi, n] = x[2g+bi][n, c*128+p]
                                  (p-major: one contiguous 4KB run/partition)
      bias   [1, 256]             (only when with_bias)
      out    [BPC, N, T]          natural layout
    """
    import concourse.bass as bass
    import concourse.mybir as mybir
    from concourse import bacc
    from concourse.tile import TileContext

    F32 = mybir.dt.float32
    # float32r: single-pass PE fp32 (TF32-ish rounding, ~1e-4 rel err) at 4x
    # the throughput of the 2-pass float32 path. PSUM accumulation stays f32.
    MMDT = mybir.dt.float32r

    nc = bacc.Bacc()
    consts = nc.declare_dram_parameter("consts", [128, 4, 256], MMDT, isOutput=False)
    xin = nc.declare_dram_parameter("xin", [NG, 128, 2, 2, N], MMDT, isOutput=False)
    if with_bias:
        bp = nc.declare_dram_parameter("bias", [1, T], F32, isOutput=False)
    out = nc.declare_dram_parameter("out", [BPC, N, T], F32, isOutput=True)

    with TileContext(nc) as tc:
        with (
            tc.tile_pool(name="const", bufs=1) as const,
            tc.tile_pool(name="xpool", bufs=4) as xpool,
            tc.tile_pool(name="hbuf", bufs=3) as hbuf,
            tc.tile_pool(name="obuf", bufs=8) as obuf,
            tc.tile_pool(name="psA", bufs=4, space="PSUM") as psA,
            tc.tile_pool(name="psB", bufs=3, space="PSUM") as psB,
            tc.tile_pool(name="psW", bufs=1, space="PSUM") as psW,
        ):
            ct = const.tile([128, 4, 256], MMDT)
            # Loads in critical-path order: W chunks -> x pair 0 (split
            # across both hwdge queues) -> AT chunks -> remaining x pairs.
            nc.sync.dma_start(out=ct[:, 0:2, :], in_=consts[:, 0:2, :])
            if with_bias:
                bias_bc = const.tile([128, T], F32)
                nc.gpsimd.dma_start(out=bias_bc, in_=bp.ap().to_broadcast([128, T]))

            # Pre-warm the PE HAM clock gate during the initial loads: a
            # memset-fed dummy matmul stream keeps PE busy so the real
            # matmuls run at 2.4 GHz from the start. Sized (~6us) to end
            # about when the first x pair lands.
            scratch = const.tile([128, 512], F32, tag="warm")
            nc.vector.memset(scratch, 0.0)
            wps = psW.tile([128, T], F32)
            for _ in range(7):
                nc.tensor.matmul(
                    wps,
                    lhsT=scratch[:, 0:128],
                    rhs=scratch[:, 0:256],
                    start=True,
                    stop=True,
                )

            xts = []
            for g in range(NG):
                xt = xpool.tile([128, 2, 2, N], MMDT)  # [p=t%128, c, bi, n]
                if g == 0:
                    # batch-0 of the first pair gets strict wire priority so
                    # its outputs (and the store stream) start ~5us earlier
                    nc.sync.dma_start(out=xt[:, 0, 0, :], in_=xin[g][:, 0, 0])
                    nc.sync.dma_start(out=xt[:, 1, 0, :], in_=xin[g][:, 1, 0])
                    nc.sync.dma_start(out=ct[:, 2:4, :], in_=consts[:, 2:4, :])
                    nc.sync.dma_start(out=xt[:, 0, 1, :], in_=xin[g][:, 0, 1])
                    nc.sync.dma_start(out=xt[:, 1, 1, :], in_=xin[g][:, 1, 1])
                else:
                    nc.sync.dma_start(out=xt, in_=xin[g])
                xts.append(xt)

            for g in range(NG):
                xt = xts[g]
                # h for the pair: [p=j%128, jc(=node block m), bi, t']
                h_sb = hbuf.tile([128, 2, 2, T], MMDT)
                if g == 0:
                    # first pair runs per-batch so batch 0's stores start as
                    # soon as its x quarter lands
                    for bi in range(2):
                        for m in range(2):
                            ph = psA.tile([128, T], F32)
                            nc.tensor.matmul(
                                ph,
                                lhsT=xt[:, 0, bi, bass.ts(m, 128)],
                                rhs=ct[:, 0, :],
                                start=True,
                                stop=False,
                            )
                            nc.tensor.matmul(
                                ph,
                                lhsT=xt[:, 1, bi, bass.ts(m, 128)],
                                rhs=ct[:, 1, :],
                                start=False,
                                stop=True,
                            )
                            nc.vector.tensor_copy(h_sb[:, m, bi, :], ph)
                        for m in range(2):
                            po = psB.tile([128, 2, T], F32)
                            nc.tensor.matmul(
                                po[:, 0, :],
                                lhsT=ct[:, 2, bass.ts(m, 128)],
                                rhs=h_sb[:, 0, bi, :],
                                start=True,
                                stop=False,
                            )
                            nc.tensor.matmul(
                                po[:, 0, :],
                                lhsT=ct[:, 3, bass.ts(m, 128)],
                                rhs=h_sb[:, 1, bi, :],
                                start=False,
                                stop=True,
                            )
                            ob = obuf.tile([128, 2, T], F32)
                            if with_bias:
                                nc.vector.tensor_add(
                                    ob[:, 0, :], po[:, 0, :], bias_bc
                                )
                            else:
                                nc.scalar.copy(out=ob[:, 0, :], in_=po[:, 0, :])
                            nc.sync.dma_start(
                                out=out[bi, bass.ts(m, 128), :], in_=ob[:, 0, :]
                            )
                    continue
                for bi in range(2):
                    for m in range(2):
                        ph = psA.tile([128, T], F32)
                        nc.tensor.matmul(
                            ph,
                            lhsT=xt[:, 0, bi, bass.ts(m, 128)],
                            rhs=ct[:, 0, :],
                            start=True,
                            stop=False,
                        )
                        nc.tensor.matmul(
                            ph,
                            lhsT=xt[:, 1, bi, bass.ts(m, 128)],
                            rhs=ct[:, 1, :],
                            start=False,
                            stop=True,
                        )
                        nc.vector.tensor_copy(h_sb[:, m, bi, :], ph)
                for m in range(2):
                    po = psB.tile([128, 2, T], F32)  # [n%128, bi, t'] one bank
                    nc.tensor.matmul(
                        po,
                        lhsT=ct[:, 2, bass.ts(m, 128)],
                        rhs=h_sb[:, 0, :, :],
                        start=True,
                        stop=False,
                    )
                    nc.tensor.matmul(
                        po,
                        lhsT=ct[:, 3, bass.ts(m, 128)],
                        rhs=h_sb[:, 1, :, :],
                        start=False,
                        stop=True,
                    )
                    ob = obuf.tile([128, 2, T], F32)
                    last = g == NG - 1
                    if with_bias:
                        for bi in range(2):
                            nc.vector.tensor_add(ob[:, bi, :], po[:, bi, :], bias_bc)
                    elif last:
                        # tail: DVE is idle and faster than ACT here
                        nc.vector.tensor_copy(ob, po)
                    else:
                        # staging copy on ACT keeps DVE free for the h copies
                        nc.scalar.copy(out=ob, in_=po)
                    dst = out[2 * g : 2 * g + 2, bass.ts(m, 128), :].rearrange(
                        "b p t -> p b t"
                    )
                    if last:
                        # split the final stores across both queues to halve
                        # the end-of-kernel drain
                        nc.sync.dma_start(out=dst[:, 0:1, :], in_=ob[:, 0:1, :])
                        nc.scalar.dma_start(out=dst[:, 1:2, :], in_=ob[:, 1:2, :])
                    else:
                        nc.sync.dma_start(out=dst, in_=ob)
    nc.finalize()
    return nc


def _ensure_axon_hooks_importable():
    """concourse's trace path hard-imports antenv.axon_hooks, which this
    image lacks. Provide the real ctypes-backed hook when possible, else a
    no-op, so BASS_TRACE=1 degrades gracefully instead of crashing."""
    try:
        import antenv.axon_hooks  # noqa: F401

        return
    except ImportError:
        pass
    try:
        import types

        import antenv

        mod = types.ModuleType("antenv.axon_hooks")
        state = {"h": None}
        mod.set_axon_ntff_profile_hook = lambda h: state.__setitem__("h", h)
        mod.get_axon_ntff_profile_hook = lambda: state["h"]
        sys.modules["antenv.axon_hooks"] = mod
        antenv.axon_hooks = mod
        try:
            from trn_agent_boot.trn_boot import _ntff_profile_via_ctypes

            hook = _ntff_profile_via_ctypes("/opt/axon/libaxon_pjrt.so")
            if hook is not None:
                mod.set_axon_ntff_profile_hook(hook)
        except Exception:
            pass
    except Exception:
        pass


def kernel(x, emb, W, b, logits, gumbel_u):
    global LAST_RESULT
    _ensure_axon_hooks_importable()
    from concourse.bass_utils import run_bass_kernel_spmd

    x = np.asarray(x, np.float32)
    W = np.asarray(W, np.float32)
    bias = np.ascontiguousarray(np.asarray(b, np.float32)).reshape(1, T)

    A = _graph_matrix(emb, logits, gumbel_u)
    # consts [128, 4, 256]: W chunks then AT chunks, partition-major
    Wr = W.reshape(2, 128, T).transpose(1, 0, 2)
    Ar = np.ascontiguousarray(A.T).reshape(2, 128, N).transpose(1, 0, 2)
    consts = np.ascontiguousarray(np.concatenate([Wr, Ar], axis=1))

    # xin [B/2 pairs, c, p, bi, n]: xT[b][t, n] split t = c*128+p, b = 2g+bi
    xT = x.transpose(0, 2, 1)  # [B, T, N]
    xpack = np.ascontiguousarray(
        xT.reshape(B // 2, 2, 2, 128, N).transpose(0, 3, 2, 1, 4)
    )

    with_bias = bool(np.any(bias))
    key = ("nc", with_bias)
    if key not in _CACHE:
        _CACHE[key] = _build_bass(with_bias)
    nc = _CACHE[key]

    in_maps = [
        {"xin": xpack[c * NG : (c + 1) * NG], "consts": consts}
        for c in range(NCORES)
    ]
    if with_bias:
        for m in in_maps:
            m["bias"] = bias
    # The first execution of a fresh NEFF occasionally trips a transient
    # NRT_EXEC_UNIT_UNRECOVERABLE; a straight retry reliably succeeds.
    last_exc = None
    for _attempt in range(3):
        try:
            res = run_bass_kernel_spmd(nc, in_maps, core_ids=list(range(NCORES)))
            break
        except Exception as e:  # noqa: BLE001
            last_exc = e
            import time as _time

            _time.sleep(2.0)
    else:
        raise last_exc
    LAST_RESULT = res
    out = np.concatenate([res.results[c]["out"] for c in range(NCORES)], axis=0)
    return out


# revision 33
# speedup vs baseline: 1.0713x; 1.0713x over previous
"""DynamicGraphEmbedding kernel for 8 Trainium2 NeuronCores.

The reference collapses algebraically:
  - deg[i] == K == 16 for every node (dst list is repeat(arange(N), K)),
    so gcn_norm edge weight ew == 1/16 for every edge.
  - straight-through gumbel gate is exactly y_hard in the forward pass,
    i.e. gate(e) = 1 iff argmax(softmax(logits[e] + g[e])) == 0.
  - therefore out[b] = A @ (x[b] @ W) + bias, with the dense [N, N] matrix
    A[i, j] = gate(i*N+j)/16 if j in topk_j[i] else 0.

Host (tiny, O(N^2)): build A from emb/logits/gumbel_u with the exact same
jax-on-CPU ops as the reference. Device (the memory-bound bulk): two chained
256^3 matmuls per batch element, data-parallel over batch across 8 cores.
"""

import sys

import numpy as np

if "/opt/trn_rl_repo" not in sys.path:
    sys.path.insert(0, "/opt/trn_rl_repo")

N, T, B, D, K = 256, 256, 64, 64, 16
NCORES = 8
BPC = B // NCORES  # batch elements per core

_CACHE = {}
LAST_RESULT = None  # BassKernelResults of the most recent run (for profiling)


def _graph_matrix(emb, logits, gumbel_u):
    """Dense [N, N] combined gate/topk/gcn-norm matrix A (host-side, tiny)."""
    try:
        import jax
        import jax.numpy as jnp

        cpu = jax.devices("cpu")[0]
        emb_j = jax.device_put(np.asarray(emb), cpu)
        logits_j = jax.device_put(np.asarray(logits), cpu)
        gu_j = jax.device_put(np.asarray(gumbel_u), cpu)
        nrm = jnp.linalg.norm(emb_j, axis=-1)
        cos = (emb_j @ emb_j.T) / (nrm[:, None] * nrm[None, :])
        _, topk_j = jax.lax.top_k(cos, K)
        g = -jnp.log(-jnp.log(gu_j))
        y_soft = jax.nn.softmax(logits_j + g, axis=-1)
        am = jnp.argmax(y_soft, axis=-1)
        topk = np.asarray(topk_j)
        gate_full = (np.asarray(am) == 0).astype(np.float32)
    except Exception:
        emb32 = np.asarray(emb, np.float32)
        nrm = np.sqrt((emb32 * emb32).sum(-1))
        cos = (emb32 @ emb32.T) / (nrm[:, None] * nrm[None, :])
        topk = np.argsort(-cos, axis=-1, kind="stable")[:, :K]
        lg = np.asarray(logits, np.float32) + np.float32(-1.0) * np.log(
            -np.log(np.asarray(gumbel_u, np.float32))
        )
        e = np.exp(lg - lg.max(-1, keepdims=True))
        y_soft = e / e.sum(-1, keepdims=True)
        gate_full = (np.argmax(y_soft, -1) == 0).astype(np.float32)
    rows = np.repeat(np.arange(N), K)
    cols = topk.reshape(-1)
    A = np.zeros((N, N), np.float32)
    A[rows, cols] = gate_full[rows * N + cols] * np.float32(0.0625)
    return A


NG = BPC // 2  # batch pairs per core


def _build_bass(with_bias):
    """Per-core Bass graph: out[b] = A @ (x[b] @ W) [+ bias] for BPC batches.

    Host-packed layouts (8KB contiguous per-partition runs, few big DMAs):
      consts [128, 4, 256]        [p, g, t]: g = (W c0, W c1, AT c0, AT c1)
      xin    [NG, 128, 2, 2, 256] [g, p, c, bi, n] = x[2g+bi][n, c*128+p]
                                  (p-major: one contiguous 4KB run/partition)
      bias   [1, 256]             (only when with_bias)
      out    [BPC, N, T]          natural layout
    """
    import concourse.bass as bass
    import concourse.mybir as mybir
    from concourse import bacc
    from concourse.tile import TileContext

    F32 = mybir.dt.float32
    # float32r: single-pass PE fp32 (TF32-ish rounding, ~1e-4 rel err) at 4x
    # the throughput of the 2-pass float32 path. PSUM accumulation stays f32.
    MMDT = mybir.dt.float32r

    nc = bacc.Bacc()
    consts = nc.declare_dram_parameter("consts", [128, 4, 256], MMDT, isOutput=False)
    xin = nc.declare_dram_parameter("xin", [NG, 128, 2, 2, N], MMDT, isOutput=False)
    if with_bias:
        bp = nc.declare_dram_parameter("bias", [1, T], F32, isOutput=False)
    out = nc.declare_dram_parameter("out", [BPC, N, T], F32, isOutput=True)

    with TileContext(nc) as tc:
        with (
            tc.tile_pool(name="const", bufs=1) as const,
            tc.tile_pool(name="xpool", bufs=4) as xpool,
            tc.tile_pool(name="hbuf", bufs=3) as hbuf,
            tc.tile_pool(name="obuf", bufs=8) as obuf,
            tc.tile_pool(name="psA", bufs=4, space="PSUM") as psA,
            tc.tile_pool(name="psB", bufs=3, space="PSUM") as psB,
            tc.tile_pool(name="psW", bufs=1, space="PSUM") as psW,
        ):
            ct = const.tile([128, 4, 256], MMDT)
            # Loads in critical-path order: W chunks -> x pair 0 (split
            # across both hwdge queues) -> AT chunks -> remaining x pairs.
            nc.sync.dma_start(out=ct[:, 0:2, :], in_=consts[:, 0:2, :])
            if with_bias:
                bias_bc = const.tile([128, T], F32)
                nc.gpsimd.dma_start(out=bias_bc, in_=bp.ap().to_broadcast([128, T]))

            # Pre-warm the PE HAM clock gate during the initial loads: a
            # memset-fed dummy matmul stream keeps PE busy so the real
            # matmuls run at 2.4 GHz from the start. Sized (~6us) to end
            # about when the first x pair lands.
            scratch = const.tile([128, 512], F32, tag="warm")
            nc.vector.memset(scratch, 0.0)
            wps = psW.tile([128, T], F32)
            for _ in range(7):
                nc.tensor.matmul(
                    wps,
                    lhsT=scratch[:, 0:128],
                    rhs=scratch[:, 0:256],
                    start=True,
                    stop=True,
                )

            xts = []
            for g in range(NG):
                xt = xpool.tile([128, 2, 2, N], MMDT)  # [p=t%128, c, bi, n]
                if g == 0:
                    # split the critical first pair across both hwdge
                    # engines so its two halves stream concurrently
                    nc.sync.dma_start(out=xt[:, 0, :, :], in_=xin[g][:, 0])
                    nc.scalar.dma_start(out=xt[:, 1, :, :], in_=xin[g][:, 1])
                else:
                    nc.sync.dma_start(out=xt, in_=xin[g])
                xts.append(xt)
                if g == 0:
                    nc.sync.dma_start(out=ct[:, 2:4, :], in_=consts[:, 2:4, :])

            for g in range(NG):
                xt = xts[g]
                # h for the pair: [p=j%128, jc(=node block m), bi, t']
                h_sb = hbuf.tile([128, 2, 2, T], MMDT)
                for bi in range(2):
                    for m in range(2):
                        ph = psA.tile([128, T], F32)
                        nc.tensor.matmul(
                            ph,
                            lhsT=xt[:, 0, bi, bass.ts(m, 128)],
                            rhs=ct[:, 0, :],
                            start=True,
                            stop=False,
                        )
                        nc.tensor.matmul(
                            ph,
                            lhsT=xt[:, 1, bi, bass.ts(m, 128)],
                            rhs=ct[:, 1, :],
                            start=False,
                            stop=True,
                        )
                        nc.vector.tensor_copy(h_sb[:, m, bi, :], ph)
                for m in range(2):
                    po = psB.tile([128, 2, T], F32)  # [n%128, bi, t'] one bank
                    nc.tensor.matmul(
                        po,
                        lhsT=ct[:, 2, bass.ts(m, 128)],
                        rhs=h_sb[:, 0, :, :],
                        start=True,
                        stop=False,
                    )
                    nc.tensor.matmul(
                        po,
                        lhsT=ct[:, 3, bass.ts(m, 128)],
                        rhs=h_sb[:, 1, :, :],
                        start=False,
                        stop=True,
                    )
                    ob = obuf.tile([128, 2, T], F32)
                    last = g == NG - 1
                    if with_bias:
                        for bi in range(2):
                            nc.vector.tensor_add(ob[:, bi, :], po[:, bi, :], bias_bc)
                    elif last:
                        # tail: DVE is idle and faster than ACT here
                        nc.vector.tensor_copy(ob, po)
                    else:
                        # staging copy on ACT keeps DVE free for the h copies
                        nc.scalar.copy(out=ob, in_=po)
                    dst = out[2 * g : 2 * g + 2, bass.ts(m, 128), :].rearrange(
                        "b p t -> p b t"
                    )
                    if last:
                        # split the final stores across both queues to halve
                        # the end-of-kernel drain
                        nc.sync.dma_start(out=dst[:, 0:1, :], in_=ob[:, 0:1, :])
                        nc.scalar.dma_start(out=dst[:, 1:2, :], in_=ob[:, 1:2, :])
                    else:
                        nc.sync.dma_start(out=dst, in_=ob)
    nc.finalize()
    return nc


def _ensure_axon_hooks_importable():
    """concourse's trace path hard-imports antenv.axon_hooks, which this
    image lacks. Provide the real ctypes-backed hook when possible, else a
    no-op, so BASS_TRACE=1 degrades gracefully instead of crashing."""
    try:
        import antenv.axon_hooks  # noqa: F401

        return
    except ImportError:
        pass
    try:
        import types

        import antenv

        mod = types.ModuleType("antenv.axon_hooks")
        state = {"h": None}
        mod.set_axon_ntff_profile_hook = lambda h: state.__setitem__("h", h)
        mod.get_axon_ntff_profile_hook = lambda: state["h"]
        sys.modules["antenv.axon_hooks"] = mod
        antenv.axon_hooks = mod
        try:
            from trn_agent_boot.trn_boot import _ntff_profile_via_ctypes

            hook = _ntff_profile_via_ctypes("/opt/axon/libaxon_pjrt.so")
            if hook is not None:
                mod.set_axon_ntff_profile_hook(hook)
        except Exception:
            pass
    except Exception:
        pass


def kernel(x, emb, W, b, logits, gumbel_u):
    global LAST_RESULT
    _ensure_axon_hooks_importable()
    from concourse.bass_utils import run_bass_kernel_spmd

    x = np.asarray(x, np.float32)
    W = np.asarray(W, np.float32)
    bias = np.ascontiguousarray(np.asarray(b, np.float32)).reshape(1, T)

    A = _graph_matrix(emb, logits, gumbel_u)
    # consts [128, 4, 256]: W chunks then AT chunks, partition-major
    Wr = W.reshape(2, 128, T).transpose(1, 0, 2)
    Ar = np.ascontiguousarray(A.T).reshape(2, 128, N).transpose(1, 0, 2)
    consts = np.ascontiguousarray(np.concatenate([Wr, Ar], axis=1))

    # xin [B/2 pairs, c, p, bi, n]: xT[b][t, n] split t = c*128+p, b = 2g+bi
    xT = x.transpose(0, 2, 1)  # [B, T, N]
    xpack = np.ascontiguousarray(
        xT.reshape(B // 2, 2, 2, 128, N).transpose(0, 3, 2, 1, 4)
    )

    with_bias = bool(np.any(bias))
    key = ("nc", with_bias)
    if key not in _CACHE:
        _CACHE[key] = _build_bass(with_bias)
    nc = _CACHE[key]

    in_maps = [
        {"xin": xpack[c * NG : (c + 1) * NG], "consts": consts}
        for c in range(NCORES)
    ]
    if with_bias:
        for m in in_maps:
            m["bias"] = bias
    # The first execution of a fresh NEFF occasionally trips a transient
    # NRT_EXEC_UNIT_UNRECOVERABLE; a straight retry reliably succeeds.
    last_exc = None
    for _attempt in range(3):
        try:
            res = run_bass_kernel_spmd(nc, in_maps, core_ids=list(range(NCORES)))
            break
        except Exception as e:  # noqa: BLE001
            last_exc = e
            import time as _time

            _time.sleep(2.0)
    else:
        raise last_exc
    LAST_RESULT = res
    out = np.concatenate([res.results[c]["out"] for c in range(NCORES)], axis=0)
    return out
